# revision 8
# baseline (speedup 1.0000x reference)
"""CTC loss on 8 Trainium2 NeuronCores (Bass/Tile).

Strategy (data parallel, per the sharding hint): batch B=64 is split 8
samples/core. Each core gathers its samples' lattice emission rows from
the (host-transposed) log-prob tensor via indirect DMA, then runs the CTC
forward recurrence in linear space:

  - per-(sample,t) max normalization (emission planes exp'd on device),
  - lattice rows computed as first-order scans over t (tensor_tensor_scan),
  - T split into C=16 chunks mapped to SBUF partitions (lanes = (b, c)),
    cross-chunk carries solved exactly with per-row transfer matrices G
    built on the PE from bulk chunk-sum cumulants,
  - per-(sample,chunk) static log offsets (host-estimated via a coarse
    windowed DP) keep all stored values in fp32 range; the stitch algebra
    folds the offsets in exactly, so they do not affect the result.

Per-sample losses are reconstructed on host from a tiny (3,128,33) output
per core (final two lattice rows + normalization cumsums): a final mean
over per-sample losses, as in the reference.
"""
import math
import numpy as np

import concourse.bass as bass
import concourse.bacc as bacc
import concourse.tile as tile
from concourse import mybir
from concourse.bass_utils import run_bass_kernel_spmd

F32 = mybir.dt.float32
I32 = mybir.dt.int32

T, B, V, S = 512, 64, 1296, 30
L = 2 * S + 1          # 61 lattice rows
C = 16                 # time chunks  (lanes = 8 local samples x 16 chunks)
TC = T // C            # 32 steps per chunk
NCORES = 8
BLOC = B // NCORES     # 8 samples per core
BLANK = 0
NEG = np.float32(-1e30)

_prog_cache = {}


# --------------------------------------------------------------------------
# host-side prep
# --------------------------------------------------------------------------

def _host_prep(log_probs, targets):
    """ext labels, allow mask, per-(b, chunk) log-level offsets Lam."""
    t2 = np.asarray(targets).reshape(B, S).astype(np.int64)
    ext = np.zeros((B, L), dtype=np.int64)
    ext[:, 1::2] = t2
    ext_m2 = np.zeros_like(ext)
    ext_m2[:, 2:] = ext[:, :-2]
    allow = ((ext != BLANK) & (ext != ext_m2)).astype(np.float32)

    # coarse per-chunk log-level estimates: windowed (blurred-emission,
    # mask-free) logsumexp DP on the gathered normalized emissions.
    lpe = np.take_along_axis(np.asarray(log_probs),
                             np.broadcast_to(ext[None], (T, B, L)), axis=2)
    m = lpe.max(axis=2)
    z = (lpe - m[:, :, None]).astype(np.float32)
    win = 2
    nw = T // win
    zw = z.reshape(nw, win, B, L).sum(axis=1) / win
    v = np.full((B, L), NEG, np.float32)
    v[:, 0] = 0.0
    v[:, 1] = 0.0
    lev = np.zeros((B, nw), np.float32)
    for i in range(nw):
        for _ in range(win):
            p1 = np.concatenate([np.full((B, 1), NEG), v[:, :-1]], axis=1)
            p2 = np.concatenate([np.full((B, 2), NEG), v[:, :-2]], axis=1)
            mx = np.maximum(np.maximum(v, p1), p2)
            s = np.exp(v - mx) + np.exp(p1 - mx) + np.exp(p2 - mx)
            v = mx + np.log(s) + zw[i]
        lev[:, i] = v.max(axis=1)
    wpc = TC // win
    Lam = np.zeros((B, C), np.float32)
    for c in range(C):
        Lam[:, c] = lev[:, c * wpc + wpc // 2]    # chunk-middle level
    return ext, allow, Lam


def _static_mats():
    """Block tri matrices over lanes (b,c): same for every core."""
    bi = np.arange(128) // C
    ci = np.arange(128) % C
    same_b = bi[:, None] == bi[None, :]
    tric = (same_b & (ci[:, None] <= ci[None, :])).astype(np.float32)
    trics = (same_b & (ci[:, None] < ci[None, :])).astype(np.float32)
    tribias = np.where(trics > 0, np.float32(0.0), NEG).astype(np.float32)
    ident = np.eye(128, dtype=np.float32)
    return tric, trics, tribias, ident


# --------------------------------------------------------------------------
# device program (identical for all cores; per-core data differs)
# --------------------------------------------------------------------------

def _build_program():
    nc = bacc.Bacc(None)
    nrows = BLOC * V * C
    lpt = nc.declare_dram_parameter("lpt", [nrows, TC], F32, isOutput=False)
    gidx = nc.declare_dram_parameter("gidx", [128, L], I32, isOutput=False)
    d_tribias = nc.declare_dram_parameter("tribias", [128, 128], F32, isOutput=False)
    d_tric = nc.declare_dram_parameter("tric", [128, 128], F32, isOutput=False)
    d_trics = nc.declare_dram_parameter("trics", [128, 128], F32, isOutput=False)
    d_ident = nc.declare_dram_parameter("ident", [128, 128], F32, isOutput=False)
    d_lam = nc.declare_dram_parameter("lam", [128, 1], F32, isOutput=False)
    d_allow2 = nc.declare_dram_parameter("allow2", [128, 29], F32, isOutput=False)
    d_e0 = nc.declare_dram_parameter("e0", [128, TC], F32, isOutput=False)
    out = nc.declare_dram_parameter("out", [3, 128, TC + 1], F32, isOutput=True)

    with tile.TileContext(nc) as tc:
        with (
            tc.tile_pool(name="consts", bufs=1) as consts,
            tc.tile_pool(name="rows", bufs=1) as rowsp,
            tc.tile_pool(name="work", bufs=3) as work,
            tc.tile_pool(name="gpool", bufs=3) as gpool,
            tc.tile_pool(name="gamp", bufs=2) as gamp,
            tc.tile_pool(name="ps", bufs=2, space="PSUM") as ps,
            tc.tile_pool(name="ps1", bufs=1, space="PSUM") as ps1,
        ):
            # ---- const loads ----
            sb_idx = consts.tile([128, L], I32)
            nc.sync.dma_start(out=sb_idx[:], in_=gidx[:])
            sb_tribias = consts.tile([128, 128], F32)
            nc.sync.dma_start(out=sb_tribias[:], in_=d_tribias[:])
            sb_tric = consts.tile([128, 128], F32)
            nc.sync.dma_start(out=sb_tric[:], in_=d_tric[:])
            sb_trics = consts.tile([128, 128], F32)
            nc.sync.dma_start(out=sb_trics[:], in_=d_trics[:])
            sb_ident = consts.tile([128, 128], F32)
            nc.sync.dma_start(out=sb_ident[:], in_=d_ident[:])
            sb_lam = consts.tile([128, 1], F32)
            nc.sync.dma_start(out=sb_lam[:], in_=d_lam[:])
            sb_allow2 = consts.tile([128, 29], F32)
            nc.sync.dma_start(out=sb_allow2[:], in_=d_allow2[:])
            sb_e0 = consts.tile([128, TC], F32)
            nc.sync.dma_start(out=sb_e0[:], in_=d_e0[:])
            sb_ones = consts.tile([1, 128], F32)
            nc.vector.memset(sb_ones[:], 1.0)
            sb_zeros = consts.tile([128, TC], F32)
            nc.vector.memset(sb_zeros[:], 0.0)

            # ---- gather emission rows: lane (b,c) row l <- lpt[gidx[lane,l]] ----
            sb_lp = consts.tile([128, L, TC], F32)
            for l in range(L):
                nc.gpsimd.indirect_dma_start(
                    out=sb_lp[:, l, :],
                    out_offset=None,
                    in_=lpt[:],
                    in_offset=bass.IndirectOffsetOnAxis(ap=sb_idx[:, l:l + 1], axis=0),
                )

            # ---- bulk normalization ----
            mmax = consts.tile([128, TC], F32)
            nc.vector.tensor_reduce(
                out=mmax[:], in_=sb_lp[:].rearrange("p l t -> p t l"),
                axis=mybir.AxisListType.X, op=mybir.AluOpType.max)
            sb_z = consts.tile([128, L, TC], F32)
            mb = mmax[:]
            m_bcast = bass.AP(tensor=mb.tensor, offset=mb.offset,
                              ap=[mb.ap[0], [0, L], mb.ap[1]])
            nc.vector.tensor_tensor(out=sb_z[:], in0=sb_lp[:], in1=m_bcast,
                                    op=mybir.AluOpType.subtract)
            sb_S = consts.tile([128, L], F32)
            nc.vector.tensor_reduce(out=sb_S[:], in_=sb_z[:],
                                    axis=mybir.AxisListType.X,
                                    op=mybir.AluOpType.add)
            sb_p = consts.tile([128, L, TC], F32)
            nc.scalar.activation(sb_p[:], sb_z[:],
                                 mybir.ActivationFunctionType.Exp)

            # ---- norm cumsums for the final reconstruction ----
            cumM = consts.tile([128, TC], F32)
            nc.vector.tensor_tensor_scan(
                out=cumM[:], data0=mmax[:], data1=sb_zeros[:], initial=0.0,
                op0=mybir.AluOpType.add, op1=mybir.AluOpType.add)
            ps_baseM = ps1.tile([128, 1], F32, tag="bulk")
            nc.tensor.matmul(out=ps_baseM[:], lhsT=sb_trics[:],
                             rhs=cumM[:, TC - 1:TC], start=True, stop=True)
            sb_baseM = consts.tile([128, 1], F32)
            nc.scalar.copy(sb_baseM[:], ps_baseM[:])

            # ---- stitch cumulants ----
            ps_lc = ps1.tile([128, L], F32, tag="bulk")
            nc.tensor.matmul(out=ps_lc[:], lhsT=sb_tric[:], rhs=sb_S[:],
                             start=True, stop=True)
            biasvec = consts.tile([128, L], F32)
            nc.vector.tensor_scalar(
                out=biasvec[:], in0=ps_lc[:], scalar1=-1.0, scalar2=sb_lam[:],
                op0=mybir.AluOpType.mult, op1=mybir.AluOpType.add)
            ps_lcs = ps1.tile([128, L], F32, tag="bulk")
            nc.tensor.matmul(out=ps_lcs[:], lhsT=sb_trics[:], rhs=sb_S[:],
                             start=True, stop=True)
            msider = consts.tile([128, L], F32)
            nc.vector.tensor_scalar(
                out=msider[:], in0=ps_lcs[:], scalar1=sb_lam[:], scalar2=None,
                op0=mybir.AluOpType.subtract)


            # ---- lattice rows ----
            row_tiles = []
            gam_prev = {}
            for l in range(L):
                # m-side row of G: transpose this row's msider column to a
                # partition-0 (1,128) stripe the matmul can take as rhs
                ps_t = ps.tile([1, 128], F32, tag="ps_t")
                nc.tensor.transpose(out=ps_t[:], in_=msider[:, l:l + 1],
                                    identity=sb_ident[:])
                stg = work.tile([1, 128], F32, tag="stg")
                nc.scalar.copy(stg[:], ps_t[:])
                psG = ps.tile([128, 128], F32, tag="psG")
                nc.tensor.matmul(out=psG[:], lhsT=sb_ones[:],
                                 rhs=stg[:], start=True, stop=False)
                nc.tensor.matmul(out=psG[:], lhsT=sb_ident[:],
                                 rhs=sb_tribias[:], start=False, stop=True)
                Gt = gpool.tile([128, 128], F32, tag="G")
                nc.scalar.activation(Gt[:], psG[:],
                                     mybir.ActivationFunctionType.Exp,
                                     bias=biasvec[:, l:l + 1])

                p_l = sb_p[:, l, :]
                if l == 0:
                    src_ap = sb_e0[:]
                elif l == 1:
                    srct = work.tile([128, TC], F32, tag="src")
                    nc.vector.tensor_add(out=srct[:],
                                         in0=row_tiles[0][:, 0:TC],
                                         in1=sb_e0[:])
                    src_ap = srct[:]
                elif l % 2 == 0:
                    src_ap = row_tiles[l - 1][:, 0:TC]
                else:
                    srct = work.tile([128, TC], F32, tag="src")
                    nc.vector.tensor_add(out=srct[:],
                                         in0=row_tiles[l - 1][:, 0:TC],
                                         in1=gam_prev[l - 2][:, 0:TC])
                    src_ap = srct[:]

                loc = work.tile([128, TC], F32, tag="loc")
                nc.vector.tensor_tensor_scan(
                    out=loc[:], data0=src_ap, data1=p_l, initial=0.0,
                    op0=mybir.AluOpType.add, op1=mybir.AluOpType.mult)
                xps = ps.tile([128, 1], F32, tag="xps")
                nc.tensor.matmul(out=xps[:], lhsT=Gt[:],
                                 rhs=loc[:, TC - 1:TC], start=True, stop=True)
                rowl = rowsp.tile([128, TC + 1], F32, tag=f"row{l}")
                nc.vector.tensor_tensor_scan(
                    out=rowl[:, 1:TC + 1], data0=src_ap, data1=p_l,
                    initial=xps[:, 0:1],
                    op0=mybir.AluOpType.add, op1=mybir.AluOpType.mult)
                nc.vector.tensor_copy(out=rowl[:, 0:1], in_=xps[:, 0:1])
                row_tiles.append(rowl)
                if l % 2 == 1 and l + 2 < L:
                    gaml = gamp.tile([128, TC + 1], F32, tag="gam")
                    nc.vector.tensor_scalar(
                        out=gaml[:], in0=rowl[:],
                        scalar1=sb_allow2[:, (l - 1) // 2:(l - 1) // 2 + 1],
                        scalar2=None, op0=mybir.AluOpType.mult)
                    gam_prev[l] = gaml

            # ---- outputs ----
            nc.sync.dma_start(out=out[0], in_=row_tiles[L - 2][:])
            nc.sync.dma_start(out=out[1], in_=row_tiles[L - 1][:])
            nc.sync.dma_start(out=out[2, :, 1:TC + 1], in_=cumM[:])
            nc.sync.dma_start(out=out[2, :, 0:1], in_=sb_baseM[:])
    nc.finalize()
    return nc


# --------------------------------------------------------------------------
# entry point
# --------------------------------------------------------------------------

def kernel(log_probs, targets, input_lengths, target_lengths):
    log_probs = np.ascontiguousarray(np.asarray(log_probs, dtype=np.float32))
    targets = np.asarray(targets)
    input_lengths = np.asarray(input_lengths).astype(np.int64)
    target_lengths = np.asarray(target_lengths)

    ext, allow, Lam = _host_prep(log_probs, targets)
    tric, trics, tribias, ident = _static_mats()

    # (T,B,V) -> (B,V,T) contiguous so each lattice row is a contiguous
    # 2KB stripe, then viewed as (B*V*C, TC) gather rows.
    lpt_all = np.ascontiguousarray(log_probs.transpose(1, 2, 0))

    bi = np.arange(BLOC).repeat(C)             # lane -> local b
    ci = np.tile(np.arange(C), BLOC)           # lane -> chunk

    if "nc" not in _prog_cache:
        _prog_cache["nc"] = _build_program()
    nc = _prog_cache["nc"]

    in_maps = []
    for k in range(NCORES):
        bsl = slice(k * BLOC, (k + 1) * BLOC)
        lpt = lpt_all[bsl].reshape(BLOC * V * C, TC)
        extk = ext[bsl]                        # (BLOC, L)
        gidx = ((bi[:, None] * V + extk[bi, :]) * C + ci[:, None]).astype(np.int32)
        lamk = Lam[bsl][bi, ci].reshape(128, 1).astype(np.float32)
        allow2 = allow[bsl][bi, :][:, 3::2].astype(np.float32)  # rows 3,5,..,59
        e0 = np.zeros((128, TC), np.float32)
        e0[ci == 0, 0] = np.exp(-Lam[bsl][bi[ci == 0], 0])
        in_maps.append({
            "lpt": lpt, "gidx": gidx,
            "tribias": tribias, "tric": tric, "trics": trics, "ident": ident,
            "lam": lamk, "allow2": np.ascontiguousarray(allow2), "e0": e0,
        })

    res = run_bass_kernel_spmd(nc, in_maps, core_ids=list(range(NCORES)))

    # host-side: per-sample loss extraction + mean (the "all-reduce")
    losses = np.zeros(B, np.float64)
    tE = input_lengths - 1
    cb, tb = tE // TC, tE % TC
    for k in range(NCORES):
        o = res.results[k]["out"]              # (3, 128, TC+1)
        for b in range(BLOC):
            gb = k * BLOC + b
            lane = b * C + cb[gb]
            A = np.float64(o[0, lane, 1 + tb[gb]]) + np.float64(o[1, lane, 1 + tb[gb]])
            lnorm = (np.float64(o[2, lane, 0]) + np.float64(o[2, lane, 1 + tb[gb]])
                     + np.float64(Lam[gb, cb[gb]]))
            lb = -(np.log(A) + lnorm) if A > 0 else np.inf
            if not np.isfinite(lb) or lb >= 1e29:
                lb = 0.0
            losses[gb] = lb
    result = np.float32(np.mean((losses / target_lengths.astype(np.float64))
                                .astype(np.float32)))
    return np.asarray(result, dtype=np.float32)


# revision 14
# speedup vs baseline: 1.0546x; 1.0546x over previous
"""CTC loss on 8 Trainium2 NeuronCores (Bass/Tile).

Strategy (data parallel, per the sharding hint): batch B=64 is split 8
samples/core. Each core gathers its samples' distinct lattice emission rows
(1 blank + 30 labels = 31 "slots" per sample) from the (host-transposed)
log-prob tensor via two indirect DMAs (full 2KB rows, one per partition),
reshuffles them into (sample, time-chunk) lanes via a DRAM bounce, then runs
the CTC forward recurrence in linear space:

  - per-(sample,t) max normalization (emission planes exp'd on device),
  - lattice rows computed as first-order scans over t (tensor_tensor_scan),
  - T split into C=16 chunks mapped to SBUF partitions (lanes = (b, c)),
    cross-chunk carries solved exactly with per-slot transfer matrices G
    built on the PE/ACT from bulk chunk-sum cumulants,
  - per-(sample,chunk) static log offsets (host-estimated via a coarse
    windowed DP) keep all stored values in fp32 range; the stitch algebra
    folds the offsets in exactly, so they do not affect the result.

Per-sample losses are reconstructed on host from a tiny (3,128,33) output
per core (final two lattice rows + normalization cumsums): a final mean
over per-sample losses, as in the reference.
"""
import numpy as np

import concourse.bass as bass
import concourse.bacc as bacc
import concourse.tile as tile
from concourse import mybir
from concourse.bass_utils import run_bass_kernel_spmd

F32 = mybir.dt.float32
I32 = mybir.dt.int32

T, B, V, S = 512, 64, 1296, 30
L = 2 * S + 1          # 61 lattice rows
NS = S + 1             # 31 distinct emission slots (slot 0 = blank)
NSP = 32               # padded slot count
C = 16                 # time chunks  (lanes = 8 local samples x 16 chunks)
TC = T // C            # 32 steps per chunk
NCORES = 8
BLOC = B // NCORES     # 8 samples per core
BLANK = 0
NEG = np.float32(-1e30)

_prog_cache = {}


def _slot(l):
    return 0 if l % 2 == 0 else (l + 1) // 2


# --------------------------------------------------------------------------
# host-side prep
# --------------------------------------------------------------------------

def _host_prep(log_probs, targets):
    """ext labels, allow mask, per-(b, chunk) log-level offsets Lam."""
    t2 = np.asarray(targets).reshape(B, S).astype(np.int64)
    ext = np.zeros((B, L), dtype=np.int64)
    ext[:, 1::2] = t2
    ext_m2 = np.zeros_like(ext)
    ext_m2[:, 2:] = ext[:, :-2]
    allow = ((ext != BLANK) & (ext != ext_m2)).astype(np.float32)

    # coarse per-chunk log-level estimates: windowed (blurred-emission,
    # mask-free) logsumexp DP on the gathered normalized emissions.
    lpe = np.take_along_axis(np.asarray(log_probs),
                             np.broadcast_to(ext[None], (T, B, L)), axis=2)
    m = lpe.max(axis=2)
    z = (lpe - m[:, :, None]).astype(np.float32)
    win = 2
    nw = T // win
    zw = z.reshape(nw, win, B, L).sum(axis=1) / win
    v = np.full((B, L), NEG, np.float32)
    v[:, 0] = 0.0
    v[:, 1] = 0.0
    lev = np.zeros((B, nw), np.float32)
    for i in range(nw):
        for _ in range(win):
            p1 = np.concatenate([np.full((B, 1), NEG), v[:, :-1]], axis=1)
            p2 = np.concatenate([np.full((B, 2), NEG), v[:, :-2]], axis=1)
            mx = np.maximum(np.maximum(v, p1), p2)
            s = np.exp(v - mx) + np.exp(p1 - mx) + np.exp(p2 - mx)
            v = mx + np.log(s) + zw[i]
        lev[:, i] = v.max(axis=1)
    wpc = TC // win
    Lam = np.zeros((B, C), np.float32)
    for c in range(C):
        Lam[:, c] = lev[:, c * wpc + wpc // 2]    # chunk-middle level
    return ext, allow, Lam


def _static_mats():
    """Block tri matrices over lanes (b,c): same for every core."""
    bi = np.arange(128) // C
    ci = np.arange(128) % C
    same_b = bi[:, None] == bi[None, :]
    tric = (same_b & (ci[:, None] <= ci[None, :])).astype(np.float32)
    trics = (same_b & (ci[:, None] < ci[None, :])).astype(np.float32)
    tribias = np.where(trics > 0, np.float32(0.0), NEG).astype(np.float32)
    ident = np.eye(128, dtype=np.float32)
    return tric, trics, tribias, ident


# --------------------------------------------------------------------------
# device program (identical for all cores; per-core data differs)
# --------------------------------------------------------------------------

def _build_program():
    nc = bacc.Bacc(None)
    lpt = nc.declare_dram_parameter("lpt", [BLOC * V, T], F32, isOutput=False)
    gidx = nc.declare_dram_parameter("gidx", [128, 2], I32, isOutput=False)
    d_tribias = nc.declare_dram_parameter("tribias", [128, 128], F32, isOutput=False)
    d_tric = nc.declare_dram_parameter("tric", [128, 128], F32, isOutput=False)
    d_trics = nc.declare_dram_parameter("trics", [128, 128], F32, isOutput=False)
    d_ident = nc.declare_dram_parameter("ident", [128, 128], F32, isOutput=False)
    d_lam = nc.declare_dram_parameter("lam", [128, 1], F32, isOutput=False)
    d_allow2 = nc.declare_dram_parameter("allow2", [128, 29], F32, isOutput=False)
    d_e0 = nc.declare_dram_parameter("e0", [128, TC], F32, isOutput=False)
    out = nc.declare_dram_parameter("out", [3, 128, TC + 1], F32, isOutput=True)
    scratch = nc.dram_tensor("scratch", [128, 2, T], F32)

    with tile.TileContext(nc) as tc:
        with (
            tc.tile_pool(name="consts", bufs=1) as consts,
            tc.tile_pool(name="rows", bufs=1) as rowsp,
            tc.tile_pool(name="work", bufs=3) as work,
            tc.tile_pool(name="gpool", bufs=3) as gpool,
            tc.tile_pool(name="gamp", bufs=2) as gamp,
            tc.tile_pool(name="ps", bufs=2, space="PSUM") as ps,
            tc.tile_pool(name="ps1", bufs=1, space="PSUM") as ps1,
        ):
            # ---- const loads ----
            sb_idx = consts.tile([128, 2], I32)
            nc.sync.dma_start(out=sb_idx[:], in_=gidx[:])
            sb_tribias = consts.tile([128, 128], F32)
            nc.sync.dma_start(out=sb_tribias[:], in_=d_tribias[:])
            sb_tric = consts.tile([128, 128], F32)
            nc.sync.dma_start(out=sb_tric[:], in_=d_tric[:])
            sb_trics = consts.tile([128, 128], F32)
            nc.sync.dma_start(out=sb_trics[:], in_=d_trics[:])
            sb_ident = consts.tile([128, 128], F32)
            nc.sync.dma_start(out=sb_ident[:], in_=d_ident[:])
            sb_lam = consts.tile([128, 1], F32)
            nc.sync.dma_start(out=sb_lam[:], in_=d_lam[:])
            sb_allow2 = consts.tile([128, 29], F32)
            nc.sync.dma_start(out=sb_allow2[:], in_=d_allow2[:])
            sb_e0 = consts.tile([128, TC], F32)
            nc.sync.dma_start(out=sb_e0[:], in_=d_e0[:])
            sb_ones = consts.tile([1, 128], F32)
            nc.vector.memset(sb_ones[:], 1.0)
            sb_zeros = consts.tile([128, TC], F32)
            nc.vector.memset(sb_zeros[:], 0.0)

            # ---- gather distinct emission rows (full 2KB rows, 2 calls) ----
            stage = consts.tile([128, 2, T + 8], F32)
            for h in range(2):
                nc.gpsimd.indirect_dma_start(
                    out=stage[:, h, 0:T], out_offset=None, in_=lpt[:],
                    in_offset=bass.IndirectOffsetOnAxis(ap=sb_idx[:, h:h + 1],
                                                        axis=0))
            # bounce through DRAM, permuting on the write so scratch holds
            # the (b, c)-lane plane layout [b, c, s, t'] directly.
            # per-sample writes; iteration (s_lo, h, c, t') both sides
            st = scratch[:]
            for b in range(BLOC):
                for h in range(2):
                    sv = stage[b * 16:(b + 1) * 16, h, 0:T]
                    w_ap = bass.AP(
                        tensor=st.tensor,
                        offset=st.offset + b * C * NSP * TC + h * 16 * TC,
                        ap=[[TC, 16], [NSP * TC, C], [1, TC]])
                    nc.sync.dma_start(out=w_ap, in_=sv)
            sb_lp = consts.tile([128, NSP, TC], F32)
            r_ap = bass.AP(
                tensor=st.tensor, offset=st.offset,
                ap=[[NSP * TC, 128], [TC, NSP], [1, TC]])
            nc.sync.dma_start(out=sb_lp[:], in_=r_ap)

            # ---- bulk normalization (slots 0..NS-1 are real) ----
            mmax = consts.tile([128, TC], F32)
            nc.vector.tensor_reduce(
                out=mmax[:], in_=sb_lp[:, 0:NS, :].rearrange("p l t -> p t l"),
                axis=mybir.AxisListType.X, op=mybir.AluOpType.max)
            sb_z = consts.tile([128, NS, TC], F32)
            mb = mmax[:]
            m_bcast = bass.AP(tensor=mb.tensor, offset=mb.offset,
                              ap=[mb.ap[0], [0, NS], mb.ap[1]])
            nc.vector.tensor_tensor(out=sb_z[:], in0=sb_lp[:, 0:NS, :],
                                    in1=m_bcast,
                                    op=mybir.AluOpType.subtract)
            sb_S = consts.tile([128, NS], F32)
            nc.vector.tensor_reduce(out=sb_S[:], in_=sb_z[:],
                                    axis=mybir.AxisListType.X,
                                    op=mybir.AluOpType.add)
            sb_p = consts.tile([128, NS, TC], F32)
            nc.scalar.activation(sb_p[:], sb_z[:],
                                 mybir.ActivationFunctionType.Exp)

            # ---- norm cumsums for the final reconstruction ----
            cumM = consts.tile([128, TC], F32)
            nc.vector.tensor_tensor_scan(
                out=cumM[:], data0=mmax[:], data1=sb_zeros[:], initial=0.0,
                op0=mybir.AluOpType.add, op1=mybir.AluOpType.add)
            ps_baseM = ps1.tile([128, 1], F32, tag="bulk")
            nc.tensor.matmul(out=ps_baseM[:], lhsT=sb_trics[:],
                             rhs=cumM[:, TC - 1:TC], start=True, stop=True)
            sb_baseM = consts.tile([128, 1], F32)
            nc.scalar.copy(sb_baseM[:], ps_baseM[:])

            # ---- stitch cumulants (per slot) ----
            ps_lc = ps1.tile([128, NS], F32, tag="bulk")
            nc.tensor.matmul(out=ps_lc[:], lhsT=sb_tric[:], rhs=sb_S[:],
                             start=True, stop=True)
            biasvec = consts.tile([128, NS], F32)
            nc.vector.tensor_scalar(
                out=biasvec[:], in0=ps_lc[:], scalar1=-1.0, scalar2=sb_lam[:],
                op0=mybir.AluOpType.mult, op1=mybir.AluOpType.add)
            ps_lcs = ps1.tile([128, NS], F32, tag="bulk")
            nc.tensor.matmul(out=ps_lcs[:], lhsT=sb_trics[:], rhs=sb_S[:],
                             start=True, stop=True)
            msider = consts.tile([128, NS], F32)
            nc.vector.tensor_scalar(
                out=msider[:], in0=ps_lcs[:], scalar1=sb_lam[:], scalar2=None,
                op0=mybir.AluOpType.subtract)

            # ---- per-slot G transfer matrices ----
            def build_G(s, pool, tag):
                ps_t = ps.tile([1, 128], F32, tag="ps_t")
                nc.tensor.transpose(out=ps_t[:], in_=msider[:, s:s + 1],
                                    identity=sb_ident[:])
                stg = work.tile([1, 128], F32, tag="stg")
                nc.scalar.copy(stg[:], ps_t[:])
                psG = ps.tile([128, 128], F32, tag="psG")
                nc.tensor.matmul(out=psG[:], lhsT=sb_ones[:],
                                 rhs=stg[:], start=True, stop=False)
                nc.tensor.matmul(out=psG[:], lhsT=sb_ident[:],
                                 rhs=sb_tribias[:], start=False, stop=True)
                Gt = pool.tile([128, 128], F32, tag=tag)
                nc.scalar.activation(Gt[:], psG[:],
                                     mybir.ActivationFunctionType.Exp,
                                     bias=biasvec[:, s:s + 1])
                return Gt

            G_blank = build_G(0, consts, "Gblank")

            # ---- lattice rows ----
            row_tiles = []
            gam_prev = {}
            for l in range(L):
                s = _slot(l)
                Gt = G_blank if s == 0 else build_G(s, gpool, "G")
                p_l = sb_p[:, s, :]
                if l == 0:
                    src_ap = sb_e0[:]
                elif l == 1:
                    srct = work.tile([128, TC], F32, tag="src")
                    nc.vector.tensor_add(out=srct[:],
                                         in0=row_tiles[0][:, 0:TC],
                                         in1=sb_e0[:])
                    src_ap = srct[:]
                elif l % 2 == 0:
                    src_ap = row_tiles[l - 1][:, 0:TC]
                else:
                    srct = work.tile([128, TC], F32, tag="src")
                    nc.vector.tensor_add(out=srct[:],
                                         in0=row_tiles[l - 1][:, 0:TC],
                                         in1=gam_prev[l - 2][:, 0:TC])
                    src_ap = srct[:]

                loc = work.tile([128, TC], F32, tag="loc")
                nc.vector.tensor_tensor_scan(
                    out=loc[:], data0=src_ap, data1=p_l, initial=0.0,
                    op0=mybir.AluOpType.add, op1=mybir.AluOpType.mult)
                xps = ps.tile([128, 1], F32, tag="xps")
                nc.tensor.matmul(out=xps[:], lhsT=Gt[:],
                                 rhs=loc[:, TC - 1:TC], start=True, stop=True)
                rowl = rowsp.tile([128, TC + 1], F32, tag=f"row{l}")
                nc.vector.tensor_tensor_scan(
                    out=rowl[:, 1:TC + 1], data0=src_ap, data1=p_l,
                    initial=xps[:, 0:1],
                    op0=mybir.AluOpType.add, op1=mybir.AluOpType.mult)
                nc.vector.tensor_copy(out=rowl[:, 0:1], in_=xps[:, 0:1])
                row_tiles.append(rowl)
                if l % 2 == 1 and l + 2 < L:
                    gaml = gamp.tile([128, TC + 1], F32, tag="gam")
                    nc.vector.tensor_scalar(
                        out=gaml[:], in0=rowl[:],
                        scalar1=sb_allow2[:, (l - 1) // 2:(l - 1) // 2 + 1],
                        scalar2=None, op0=mybir.AluOpType.mult)
                    gam_prev[l] = gaml

            # ---- outputs ----
            nc.sync.dma_start(out=out[0], in_=row_tiles[L - 2][:])
            nc.sync.dma_start(out=out[1], in_=row_tiles[L - 1][:])
            nc.sync.dma_start(out=out[2, :, 1:TC + 1], in_=cumM[:])
            nc.sync.dma_start(out=out[2, :, 0:1], in_=sb_baseM[:])
    nc.finalize()
    return nc


# --------------------------------------------------------------------------
# entry point
# --------------------------------------------------------------------------

def kernel(log_probs, targets, input_lengths, target_lengths):
    log_probs = np.ascontiguousarray(np.asarray(log_probs, dtype=np.float32))
    targets = np.asarray(targets)
    input_lengths = np.asarray(input_lengths).astype(np.int64)
    target_lengths = np.asarray(target_lengths)

    ext, allow, Lam = _host_prep(log_probs, targets)
    tric, trics, tribias, ident = _static_mats()

    # (T,B,V) -> (B,V,T) contiguous so each lattice row is a contiguous
    # 2KB stripe; per-core view (BLOC*V, T).
    lpt_all = np.ascontiguousarray(log_probs.transpose(1, 2, 0))
    t2 = targets.reshape(B, S).astype(np.int64)
    vrows = np.zeros((B, NS), np.int64)
    vrows[:, 1:] = t2                      # slot s>=1 -> label s-1; slot 0 = blank

    bi = np.arange(BLOC).repeat(C)             # lane -> local b
    ci = np.tile(np.arange(C), BLOC)           # lane -> chunk

    if "nc" not in _prog_cache:
        _prog_cache["nc"] = _build_program()
    nc = _prog_cache["nc"]

    in_maps = []
    for k in range(NCORES):
        bsl = slice(k * BLOC, (k + 1) * BLOC)
        lpt = lpt_all[bsl].reshape(BLOC * V, T)
        # gather indices: call h, partition pi=(b*16+j) -> slot s=h*16+j
        gidx = np.zeros((128, 2), np.int32)
        pb = np.arange(128) // 16
        pj = np.arange(128) % 16
        for h in range(2):
            s = np.minimum(h * 16 + pj, NS - 1)
            gidx[:, h] = (pb * V + vrows[bsl][pb, s]).astype(np.int32)
        lamk = Lam[bsl][bi, ci].reshape(128, 1).astype(np.float32)
        allow2 = allow[bsl][bi, :][:, 3::2].astype(np.float32)  # rows 3,5,..,59
        e0 = np.zeros((128, TC), np.float32)
        e0[ci == 0, 0] = np.exp(-Lam[bsl][bi[ci == 0], 0])
        in_maps.append({
            "lpt": lpt, "gidx": gidx,
            "tribias": tribias, "tric": tric, "trics": trics, "ident": ident,
            "lam": lamk, "allow2": np.ascontiguousarray(allow2), "e0": e0,
        })

    res = run_bass_kernel_spmd(nc, in_maps, core_ids=list(range(NCORES)))

    # host-side: per-sample loss extraction + mean (the "all-reduce")
    losses = np.zeros(B, np.float64)
    tE = input_lengths - 1
    cb, tb = tE // TC, tE % TC
    for k in range(NCORES):
        o = res.results[k]["out"]              # (3, 128, TC+1)
        for b in range(BLOC):
            gb = k * BLOC + b
            lane = b * C + cb[gb]
            A = np.float64(o[0, lane, 1 + tb[gb]]) + np.float64(o[1, lane, 1 + tb[gb]])
            lnorm = (np.float64(o[2, lane, 0]) + np.float64(o[2, lane, 1 + tb[gb]])
                     + np.float64(Lam[gb, cb[gb]]))
            lb = -(np.log(A) + lnorm) if A > 0 else np.inf
            if not np.isfinite(lb) or lb >= 1e29:
                lb = 0.0
            losses[gb] = lb
    result = np.float32(np.mean((losses / target_lengths.astype(np.float64))
                                .astype(np.float32)))
    return np.asarray(result, dtype=np.float32)
